# revision 27
# baseline (speedup 1.0000x reference)
"""Two-layer GraphSAGE (mean aggr) on 8 Trainium2 NeuronCores.

Strategy (1D graph partitioning by destination node):
  - Core k owns dst nodes [k*NPC, (k+1)*NPC) and all edges into them.
  - Aggregation per 128-node "bin": gather the source rows of the bin's
    edges with dma_gather (one 256B bf16 row per edge), build a per-chunk
    weighted one-hot indicator on DVE (iota is_equal dstcol, * 1/deg),
    and accumulate G.T @ ind into PSUM on the TensorEngine. The PSUM
    tile is the transposed, mean-normalized aggregation for the bin.
  - h = relu(aggT.T @ W1l + xT.T @ W1r + 1.T @ b1) per bin; h is written
    to a local DRAM shard and AllGathered into a Shared DRAM table that
    layer 2's gathers read (sources span all cores).
  - Everything except the final output and PSUM accumulation is bf16:
    gather tables (halves SBUF + HBM bytes), indicators (2x DVE), and
    matmul operands (4x PE vs fp32).
  - dma_gather indices are int16, so each gather table is split at row
    LOSPLIT=32768 (lo/hi); per-bin edge lists are sorted lo-then-hi and
    chunked into 128-edge chunks (padded with weight-0 edges).
  - All per-core variation is input data (indices / columns / weights);
    the NEFF is one SPMD program. Chunk counts (K_lo/K_hi per layer) are
    derived from the actual graph at call time, then compiled.
"""

from dataclasses import dataclass

import numpy as np


@dataclass(frozen=True)
class Cfg:
    n_nodes: int = 50000
    d_in: int = 96
    d_hid: int = 128
    d_out: int = 128
    nc: int = 8
    lo_split: int = 32768
    call_ch: int = 8         # 128-edge chunks per dma_gather call
    n_queues: int = 4        # SWDGE queues (parallel Q7 descriptor gen)
    g_bufs: int = 12         # in-flight gather tiles
    reps: int = 1            # repeat whole computation in-NEFF (timing only)

    @property
    def npc(self):
        return self.n_nodes // self.nc

    @property
    def bins(self):
        return -(-self.npc // 128)

    @property
    def seg(self):
        return self.bins * 128

    @property
    def tbl(self):
        return self.seg * self.nc


DEFAULT_CFG = Cfg()


def _bf16(a):
    import ml_dtypes
    return np.asarray(a, dtype=np.float32).astype(ml_dtypes.bfloat16)


# ---------------------------------------------------------------- host side

def _wrap16(a):
    """Gather index layout: idx i -> [i % 16, i // 16], replicated 8x
    across the 128 partitions (one copy per Q7 core)."""
    return np.tile(a.reshape(-1, 16).T, (8, 1))


def _per_core_chunks(cfg, src_ids, cols, ws, bins, k_lo, k_hi):
    """Arrange one core's edges (already split per bin) into the fixed
    chunk structure: per bin, k_lo lo-chunks then k_hi hi-chunks of 128
    edges. Returns (idx int16 [NCH*128], dc f32 [NCH*128]). idx is
    stream-major (lo block of bins*k_lo chunks, then the hi block); dc
    is bin-major (bin b's k_lo+k_hi chunks contiguous). Padding slots
    get idx 0 and dc -1 (no indicator match)."""
    n_bins = cfg.bins
    nk = k_lo + k_hi
    nch = n_bins * nk
    idx = np.zeros(nch * 128, dtype=np.int16)
    dc = np.full(nch * 128, -1.0, dtype=np.float32)
    w = np.zeros(nch * 128, dtype=np.float32)
    n_lo_ch = n_bins * k_lo
    order = np.argsort(bins, kind="stable")
    src_ids, cols, ws, bins = (src_ids[order], cols[order], ws[order],
                               bins[order])
    bounds = np.searchsorted(bins, np.arange(n_bins + 1))
    for b in range(n_bins):
        s = src_ids[bounds[b]:bounds[b + 1]]
        c = cols[bounds[b]:bounds[b + 1]]
        wt = ws[bounds[b]:bounds[b + 1]]
        lo = s < cfg.lo_split
        for is_lo, kcap, idx_ch, dc_ch in (
                (True, k_lo, b * k_lo, b * nk),
                (False, k_hi, n_lo_ch + b * k_hi, b * nk + k_lo)):
            ss = s[lo] if is_lo else s[~lo] - cfg.lo_split
            cc = c[lo] if is_lo else c[~lo]
            ww = wt[lo] if is_lo else wt[~lo]
            # sort by source id for DRAM page locality in the gather
            so = np.argsort(ss, kind="stable")
            ss, cc, ww = ss[so], cc[so], ww[so]
            assert len(ss) <= kcap * 128, (len(ss), kcap)
            o = idx_ch * 128
            idx[o:o + len(ss)] = ss.astype(np.int16)
            o = dc_ch * 128
            dc[o:o + len(cc)] = cc.astype(np.float32)
            w[o:o + len(ww)] = ww.astype(np.float32)
    return idx, dc, w


def _balance_bins(cfg, loads):
    """Assign local dst nodes to bins (<=128 each), balancing the four
    per-bin chunk loads (l1lo, l1hi, l2lo, l2hi). Greedy: heaviest node
    first into the feasible bin with the smallest resulting max-axis
    load. Returns (bin_of_local, col_of_local)."""
    n_bins, npc = cfg.bins, cfg.npc
    tot = loads.sum(axis=1)
    order = np.argsort(-tot, kind="stable")
    acc = np.zeros((n_bins, loads.shape[1]), dtype=np.int64)
    cnt = np.zeros(n_bins, dtype=np.int64)
    bin_of = np.zeros(npc, dtype=np.int64)
    # normalize each axis by its mean per-bin load so maxima compare
    scale = 1.0 / np.maximum(loads.sum(axis=0) / n_bins, 1.0)
    for n in order:
        cand = (acc + loads[n]) * scale
        score = cand.max(axis=1)
        score[cnt >= 128] = np.inf
        b = int(np.argmin(score))
        bin_of[n] = b
        acc[b] += loads[n]
        cnt[b] += 1
    col_of = np.zeros(npc, dtype=np.int64)
    nxt = np.zeros(n_bins, dtype=np.int64)
    for n in range(npc):
        b = bin_of[n]
        col_of[n] = nxt[b]
        nxt[b] += 1
    return bin_of, col_of


def preprocess(cfg, x, edge_index, W1l, b1, W1r, W2l, b2, W2r):
    src = np.asarray(edge_index[0], dtype=np.int64)
    dst = np.asarray(edge_index[1], dtype=np.int64)
    x = np.asarray(x, dtype=np.float32)
    npc, n_bins = cfg.npc, cfg.bins

    deg = np.bincount(dst, minlength=cfg.n_nodes).astype(np.float32)
    inv_deg = (1.0 / np.maximum(deg, 1.0)).astype(np.float32)
    w_edge = inv_deg[dst]

    owner = dst // npc
    local = dst - owner * npc
    src_loc = src - (src // npc) * npc

    # Balanced bin assignment per core. l2lo/l2hi depend on the h-table
    # position of the SOURCE, which depends on the source owner's own bin
    # assignment -- approximate with the l1 split (same 0.66/0.34 ratio):
    # the l2 split of a source at local position p is pos < 32768, i.e.
    # roughly source owner < 5, nearly the same edge set as src < 32768.
    bin_of = np.empty((cfg.nc, npc), dtype=np.int64)
    col_of = np.empty((cfg.nc, npc), dtype=np.int64)
    lo1_mask = src < cfg.lo_split
    lo2_mask = src // npc < 5   # owner 0-4 <=> pos < 5*seg = lo_split
    for k in range(cfg.nc):
        sel = owner == k
        l_k = local[sel]
        loads = np.zeros((npc, 4), dtype=np.int64)
        np.add.at(loads[:, 0], l_k[lo1_mask[sel]], 1)
        np.add.at(loads[:, 1], l_k[~lo1_mask[sel]], 1)
        np.add.at(loads[:, 2], l_k[lo2_mask[sel]], 1)
        np.add.at(loads[:, 3], l_k[~lo2_mask[sel]], 1)
        bin_of[k], col_of[k] = _balance_bins(cfg, loads)

    ebin = bin_of[owner, local]
    ecol = col_of[owner, local]

    sowner = src // npc
    pos = (cfg.seg * sowner + bin_of[sowner, src_loc] * 128
           + col_of[sowner, src_loc])      # row of src in h table

    per_core = []
    k1lo = k1hi = k2lo = k2hi = 1
    for k in range(cfg.nc):
        sel = owner == k
        s_k, b_k, c_k, w_k = src[sel], ebin[sel], ecol[sel], w_edge[sel]
        p_k = pos[sel]
        per_core.append((s_k, p_k, b_k, c_k, w_k))
        cnt = np.bincount(b_k, minlength=n_bins)
        lo1 = np.bincount(b_k[s_k < cfg.lo_split], minlength=n_bins)
        lo2 = np.bincount(b_k[p_k < cfg.lo_split], minlength=n_bins)
        k1lo = max(k1lo, int(np.max(-(-lo1 // 128))))
        k1hi = max(k1hi, int(np.max(-(-(cnt - lo1) // 128))))
        k2lo = max(k2lo, int(np.max(-(-lo2 // 128))))
        k2hi = max(k2hi, int(np.max(-(-(cnt - lo2) // 128))))

    xpad = np.zeros((cfg.n_nodes, 128), dtype=np.float32)
    xpad[:, :cfg.d_in] = x
    xpad_bf = _bf16(xpad)
    iota = np.tile(np.arange(128, dtype=np.float32), (128, 1))
    ident = np.eye(128, dtype=np.float32)

    in_maps = []
    for k in range(cfg.nc):
        s_k, p_k, b_k, c_k, w_k = per_core[k]
        i1, d1, w1 = _per_core_chunks(cfg, s_k, c_k, w_k, b_k, k1lo, k1hi)
        i2, d2, w2 = _per_core_chunks(cfg, p_k, c_k, w_k, b_k, k2lo, k2hi)
        xT = np.zeros((cfg.d_in, cfg.seg), dtype=np.float32)
        xT[:, bin_of[k] * 128 + col_of[k]] = x[k * npc:(k + 1) * npc].T
        in_maps.append({
            "xpad": xpad_bf,
            "xT": _bf16(xT),
            "idx1": np.ascontiguousarray(_wrap16(i1)),
            "dc1": _bf16(np.ascontiguousarray(d1.reshape(-1, 128).T)),
            "w1": _bf16(np.ascontiguousarray(w1.reshape(-1, 128).T)),
            "idx2": np.ascontiguousarray(_wrap16(i2)),
            "dc2": _bf16(np.ascontiguousarray(d2.reshape(-1, 128).T)),
            "w2": _bf16(np.ascontiguousarray(w2.reshape(-1, 128).T)),
            "W1l": _bf16(W1l),
            "W1r": _bf16(W1r),
            "W2l": _bf16(W2l),
            "W2r": _bf16(W2r),
            "b1row": _bf16(np.asarray(b1, np.float32).reshape(1, cfg.d_hid)),
            "b2row": _bf16(np.asarray(b2, np.float32).reshape(1, cfg.d_out)),
            "iota": _bf16(iota),
            "ident": _bf16(ident),
        })
    perm = bin_of * 128 + col_of     # [nc, npc]: local node -> out row
    return in_maps, (k1lo, k1hi, k2lo, k2hi), perm


# -------------------------------------------------------------- device side

def build_program(cfg, k1lo, k1hi, k2lo, k2hi, debug_parts=frozenset()):
    """debug_parts: subset of {"no_l2", "no_gather", "no_mm"} to stub out
    components when bisecting hardware failures."""
    import concourse.bacc as bacc
    import concourse.tile as tile
    import concourse.mybir as mybir

    F32 = mybir.dt.float32
    BF16 = mybir.dt.bfloat16
    I16 = mybir.dt.int16
    EQ = mybir.AluOpType.is_equal
    MUL = mybir.AluOpType.mult

    n_bins, seg, tbl = cfg.bins, cfg.seg, cfg.tbl
    d_in, d_hid, d_out = cfg.d_in, cfg.d_hid, cfg.d_out
    nch1 = n_bins * (k1lo + k1hi)
    nch2 = n_bins * (k2lo + k2hi)

    nc = bacc.Bacc("TRN2", target_bir_lowering=False, debug=False,
                   num_devices=cfg.nc, num_swdge_queues=cfg.n_queues)

    xpad = nc.dram_tensor("xpad", [cfg.n_nodes, 128], BF16,
                          kind="ExternalInput")
    xT_d = nc.dram_tensor("xT", [d_in, seg], BF16, kind="ExternalInput")
    idx1_d = nc.dram_tensor("idx1", [128, nch1 * 8], I16, kind="ExternalInput")
    dc1_d = nc.dram_tensor("dc1", [128, nch1], BF16, kind="ExternalInput")
    w1_d = nc.dram_tensor("w1", [128, nch1], BF16, kind="ExternalInput")
    idx2_d = nc.dram_tensor("idx2", [128, nch2 * 8], I16, kind="ExternalInput")
    dc2_d = nc.dram_tensor("dc2", [128, nch2], BF16, kind="ExternalInput")
    w2_d = nc.dram_tensor("w2", [128, nch2], BF16, kind="ExternalInput")
    W1l_d = nc.dram_tensor("W1l", [d_in, d_hid], BF16, kind="ExternalInput")
    W1r_d = nc.dram_tensor("W1r", [d_in, d_hid], BF16, kind="ExternalInput")
    W2l_d = nc.dram_tensor("W2l", [d_hid, d_out], BF16, kind="ExternalInput")
    W2r_d = nc.dram_tensor("W2r", [d_hid, d_out], BF16, kind="ExternalInput")
    b1_d = nc.dram_tensor("b1row", [1, d_hid], BF16, kind="ExternalInput")
    b2_d = nc.dram_tensor("b2row", [1, d_out], BF16, kind="ExternalInput")
    iota_d = nc.dram_tensor("iota", [128, 128], BF16, kind="ExternalInput")
    ident_d = nc.dram_tensor("ident", [128, 128], BF16, kind="ExternalInput")
    out_d = nc.dram_tensor("out", [seg, d_out], BF16, kind="ExternalOutput")

    h_cc_in = nc.dram_tensor("h_cc_in", [seg, d_hid], BF16)
    h_full = nc.dram_tensor("h_full", [tbl, d_hid], BF16, addr_space="Shared")

    with tile.TileContext(nc) as tc:
        with tc.tile_pool(name="const", bufs=1) as cp, \
             tc.tile_pool(name="work", bufs=6) as wp, \
             tc.tile_pool(name="gtiles", bufs=cfg.g_bufs) as gp, \
             tc.tile_pool(name="ind", bufs=4) as ip, \
             tc.tile_pool(name="psA", bufs=4, space="PSUM") as psA, \
             tc.tile_pool(name="psB", bufs=3, space="PSUM") as psB, \
             tc.tile_pool(name="psC", bufs=1, space="PSUM") as psC:

            # ---- persistent constants
            xT = cp.tile([d_in, seg], BF16)
            hT = cp.tile([d_hid, seg], BF16)
            W1l = cp.tile([d_in, d_hid], BF16)
            W1r = cp.tile([d_in, d_hid], BF16)
            W2l = cp.tile([d_hid, d_out], BF16)
            W2r = cp.tile([d_hid, d_out], BF16)
            b1r = cp.tile([1, d_hid], BF16)
            b2r = cp.tile([1, d_out], BF16)
            ones = cp.tile([1, 128], BF16)
            iota = cp.tile([128, 128], BF16)
            ident = cp.tile([128, 128], BF16)
            idx1 = cp.tile([128, nch1 * 8], I16)
            dc1 = cp.tile([128, nch1], BF16)
            w1 = cp.tile([128, nch1], BF16)
            idx2 = cp.tile([128, nch2 * 8], I16)
            dc2 = cp.tile([128, nch2], BF16)
            w2 = cp.tile([128, nch2], BF16)

            for t, d in ((xT, xT_d), (W1l, W1l_d), (W1r, W1r_d),
                         (W2l, W2l_d), (W2r, W2r_d), (b1r, b1_d),
                         (b2r, b2_d), (iota, iota_d), (ident, ident_d),
                         (dc1, dc1_d), (w1, w1_d), (dc2, dc2_d),
                         (w2, w2_d)):
                nc.sync.dma_start(t[:], d[:])
            # idx arrays on gpsimd so the gathers see them in program order
            # (idx2 is loaded later, after layer 1, off the startup path)
            nc.gpsimd.dma_start(idx1[:], idx1_d[:])
            nc.vector.memset(ones[:], 1.0)
            dummy_g = cp.tile([128, 1, 128], BF16)
            nc.vector.memset(dummy_g[:], 0.5)

            call_state = {"n": 0}

            def layer(klo, khi, idx_t, dc_t, w_t, lo_src, hi_src, df,
                      consume, tag, after_bin=None):
                n_lo_ch = n_bins * klo
                nk = klo + khi
                n_ch_stream = {0: n_lo_ch, 1: n_bins * khi}
                src = {0: lo_src, 1: hi_src}
                idx_base = {0: 0, 1: n_lo_ch}
                tiles = {}

                def chunk_tile(s, c):
                    """Gather tile slice holding stream-s chunk c, issuing
                    the covering dma_gather call on first touch."""
                    if "no_gather" in debug_parts:
                        return dummy_g[:, 0, :]
                    call = c // cfg.call_ch
                    if (s, call) not in tiles:
                        c0 = call * cfg.call_ch
                        n_c = min(cfg.call_ch, n_ch_stream[s] - c0)
                        G = gp.tile([128, cfg.call_ch, 128], BF16, tag="G")
                        q = call_state["n"] % cfg.n_queues
                        call_state["n"] += 1
                        ic0 = (idx_base[s] + c0) * 8
                        nc.gpsimd.dma_gather(
                            out_ap=G[:, 0:n_c, :], in_ap=src[s],
                            idxs_ap=idx_t[:, ic0:ic0 + n_c * 8],
                            num_idxs=n_c * 128, num_idxs_reg=n_c * 128,
                            elem_size=128, single_packet=False,
                            queue_num=q)
                        tiles[(s, call)] = G
                    return tiles[(s, call)][:, c - call * cfg.call_ch, :]

                for b in range(n_bins):
                    # one wide weighted-one-hot indicator for the bin's nk
                    # chunks: wind[p, ci, j] = (j == dc[p, b*nk+ci]) * w
                    wind = ip.tile([128, nk, 128], BF16, tag="wind")
                    nc.vector.tensor_tensor(
                        out=wind[:],
                        in0=iota[:].unsqueeze(1).broadcast_to((128, nk, 128)),
                        in1=dc_t[:, b * nk:(b + 1) * nk]
                            .broadcast_to((128, nk, 128)),
                        op=EQ)
                    nc.vector.tensor_tensor(
                        out=wind[:], in0=wind[:],
                        in1=w_t[:, b * nk:(b + 1) * nk]
                            .broadcast_to((128, nk, 128)),
                        op=MUL)
                    pagg = psA.tile([df, 128], F32, tag="pagg")
                    nmm = 1 if "no_mm" in debug_parts else nk
                    for ci in range(nmm):
                        if ci < klo:
                            s, c = 0, b * klo + ci
                        else:
                            s, c = 1, b * khi + (ci - klo)
                        g_sl = chunk_tile(s, c)[:, 0:df]
                        nc.tensor.matmul(pagg[:], lhsT=g_sl,
                                         rhs=wind[:, ci, :],
                                         start=(ci == 0),
                                         stop=(ci == nmm - 1))
                    consume(b, pagg)
                    if after_bin is not None:
                        after_bin(b)

            # ---------------- layer 1
            def consume1(b, pagg):
                aggT = wp.tile([d_in, 128], BF16, tag="aggT")
                nc.scalar.copy(aggT[:], pagg[:])
                ph = psB.tile([128, d_hid], F32, tag="ph")
                nc.tensor.matmul(ph[:], lhsT=aggT[:], rhs=W1l[:],
                                 start=True, stop=False)
                nc.tensor.matmul(ph[:], lhsT=xT[:, b * 128:(b + 1) * 128],
                                 rhs=W1r[:], start=False, stop=False)
                nc.tensor.matmul(ph[:], lhsT=ones[:], rhs=b1r[:],
                                 start=False, stop=True)
                h_t = wp.tile([128, d_hid], BF16, tag="h")
                nc.vector.tensor_scalar_max(h_t[:], ph[:], 0.0)
                nc.sync.dma_start(h_cc_in[b * 128:(b + 1) * 128, :], h_t[:])
                ptr = psC.tile([128, 128], BF16, tag="ptr")
                nc.tensor.transpose(ptr[:], h_t[:], ident[:])
                nc.scalar.copy(hT[:, b * 128:(b + 1) * 128], ptr[:])

            # ---------------- layer 2
            def consume2(b, pagg):
                agg2T = wp.tile([d_hid, 128], BF16, tag="agg2T")
                nc.scalar.copy(agg2T[:], pagg[:])
                po = psB.tile([128, d_out], F32, tag="ph")
                nc.tensor.matmul(po[:], lhsT=agg2T[:], rhs=W2l[:],
                                 start=True, stop=False)
                nc.tensor.matmul(po[:], lhsT=hT[:, b * 128:(b + 1) * 128],
                                 rhs=W2r[:], start=False, stop=False)
                nc.tensor.matmul(po[:], lhsT=ones[:], rhs=b2r[:],
                                 start=False, stop=True)
                o_t = wp.tile([128, d_out], BF16, tag="o")
                nc.scalar.copy(o_t[:], po[:])
                nc.sync.dma_start(out_d[b * 128:(b + 1) * 128, :], o_t[:])

            for _rep in range(cfg.reps):
                layer(k1lo, k1hi, idx1, dc1, w1, xpad[0:cfg.lo_split, :],
                      xpad[cfg.lo_split:cfg.n_nodes, :], d_in, consume1, "1")

                nc.gpsimd.dma_start(idx2[:], idx2_d[:])
                # ---- AllGather h
                nc.gpsimd.collective_compute(
                    "AllGather", mybir.AluOpType.bypass,
                    replica_groups=[list(range(cfg.nc))],
                    ins=[h_cc_in[:]], outs=[h_full[:]])
                bounce = wp.tile([1, d_hid], BF16, tag="bounce")
                nc.gpsimd.dma_start(bounce[:], h_full[0:1, :])

                if "no_l2" in debug_parts:
                    for b in range(n_bins):
                        # just copy hT slices out to keep deps simple
                        o_t = wp.tile([128, d_out], BF16, tag="o")
                        nc.vector.tensor_copy(o_t[:],
                                              hT[:, b * 128:(b + 1) * 128])
                        nc.sync.dma_start(out_d[b * 128:(b + 1) * 128, :],
                                          o_t[:])
                else:
                    layer(k2lo, k2hi, idx2, dc2, w2,
                          h_full[0:cfg.lo_split, :],
                          h_full[cfg.lo_split:tbl, :], d_hid, consume2, "2")

    nc.compile()
    return nc


_CACHE = {}


def run(cfg, inputs, _want_results=False, **spmd_kwargs):
    from concourse.bass_utils import run_bass_kernel_spmd

    in_maps, ks, perm = preprocess(cfg, **inputs)
    key = (cfg, ks)
    if key not in _CACHE:
        _CACHE[key] = build_program(cfg, *ks)
    nc = _CACHE[key]
    res = run_bass_kernel_spmd(nc, in_maps, core_ids=list(range(cfg.nc)),
                               **spmd_kwargs)
    npc = cfg.npc
    out = np.empty((cfg.n_nodes, cfg.d_out), dtype=np.float32)
    for k in range(cfg.nc):
        out[k * npc:(k + 1) * npc] = np.asarray(
            res.results[k]["out"], dtype=np.float32)[perm[k]]
    if _want_results:
        return out, res
    return out


def kernel(x, edge_index, W1l, b1, W1r, W2l, b2, W2r):
    return run(DEFAULT_CFG, dict(x=x, edge_index=edge_index, W1l=W1l, b1=b1,
                                 W1r=W1r, W2l=W2l, b2=b2, W2r=W2r))
